# revision 2
# baseline (speedup 1.0000x reference)
"""CAM (channel attention) module kernel for Trainium2, 8-core data-parallel.

Computes, per batch b (one batch per NeuronCore):
    q = x[b].reshape(C, N)                  # C=512, N=4096
    E = q @ q.T                             # [C, C], symmetric
    att = softmax(rowmax(E) - E, axis=-1)   # == exp(rowmin(E)-E)/rowsum
    out = gamma * (att @ q) + x[b]

v3 design (v2 lost ~25us to a serialized xbar-transpose tail, PE idling
during the load phase, and gpsimd teardown):
  - all qT tiles built with PE transposes (xbar only for attT i>=1); the
    PE has idle slack while the 8MB x load streams, so the transposes are
    free, and the energy tail starts the moment the last chunk lands.
  - load in 20 chunks (12x1024 + 8x512 cols); the last k-slab is only 4
    k-tiles wide so the post-load energy tail for row-block 0 is ~1us.
  - tail: E-tail(i) staggered i-outer, mirrors E[i,j<i] from E[j] via PE
    transpose, softmax chains overlap on DVE/ACT, attT(0) on PE (fast
    path to the first out matmul), attT(1..3) on the DMA xbar.
  - out phase: per 512-col chunk, 4 bf16 matmuls into a 1-bank PSUM tile
    (pool bufs=2) + DVE scalar_tensor_tensor out=(psum*gamma/s)+x, store
    on alternating HWDGE rings. PE-bound at ~27.6us.
  - no gpsimd anywhere: identity matrices and a pre-broadcast gamma are
    supplied as extra host inputs, stores ride sync/scalar only.
  - att left unnormalized; gamma/s rides the final DVE op, so gamma=0
    gives out == x exactly (fp32 x add).
"""

import sys

import numpy as np

for _p in ("/opt/trn_rl_repo",):
    if _p not in sys.path:
        sys.path.insert(0, _p)

B, C, H, W = 8, 512, 64, 64
N = H * W  # 4096
P = 128
CT = C // P  # 4 channel tiles
KT = N // P  # 32 spatial tiles
FD = 512  # matmul free-dim / PSUM bank width (fp32)

_CACHE = {}


def _build_bass():
    import concourse.mybir as mybir
    import concourse.tile as tile
    from concourse import bacc

    fp32 = mybir.dt.float32
    bf16 = mybir.dt.bfloat16
    AX = mybir.AxisListType.X
    ALU = mybir.AluOpType
    ACT_EXP = mybir.ActivationFunctionType.Exp

    nc = bacc.Bacc(None, target_bir_lowering=False, debug=False)
    x_d = nc.dram_tensor("x", [C, N], fp32, kind="ExternalInput")
    g_d = nc.dram_tensor("gammab", [P, 1], fp32, kind="ExternalInput")
    idb_d = nc.dram_tensor("identb", [P, P], bf16, kind="ExternalInput")
    idf_d = nc.dram_tensor("identf", [P, P], fp32, kind="ExternalInput")
    o_d = nc.dram_tensor("out", [C, N], fp32, kind="ExternalOutput")

    # load chunks: (c_tile, col0, width); 1024-wide for k-slabs 0..23,
    # 512-wide for the last two half-slabs (k 24..27, 28..31)
    chunks = [(c, h * 1024, 1024) for h in range(3) for c in range(CT)]
    chunks += [(c, 3072 + s * 512, 512) for s in range(2) for c in range(CT)]

    with tile.TileContext(nc) as tc:
        with (
            tc.tile_pool(name="persist", bufs=1) as persist,
            tc.tile_pool(name="stats", bufs=4) as stats,
            tc.tile_pool(name="rgp", bufs=4) as rgp,
            tc.tile_pool(name="outp", bufs=4) as outp,
            tc.tile_pool(name="epsum", bufs=4, space="PSUM") as epsum,
            tc.tile_pool(name="opsum", bufs=2, space="PSUM") as opsum,
            tc.tile_pool(name="atps", bufs=2, space="PSUM") as atps,
        ):
            gam = persist.tile([P, 1], fp32)
            ident = persist.tile([P, P], bf16)
            ident32 = persist.tile([P, P], fp32)
            q = persist.tile([P, CT, N], fp32)
            q_bf = persist.tile([P, CT, N], bf16)
            # k-major qT: qT[p, k, c, v] = q[c*128+v, k*128+p]; energy rhs for
            # chunk k is the contiguous [128, 512] slab qT[:, k, :, :]
            qT = persist.tile([P, KT, CT, P], bf16)
            att = persist.tile([P, CT, C], bf16)
            attT = persist.tile([P, CT, CT, P], bf16)

            # issue every input DMA up front: both HWDGE rings stream
            # back-to-back with no interleaved non-load traffic.
            nc.sync.dma_start(out=gam, in_=g_d[:, :])
            nc.scalar.dma_start(out=ident, in_=idb_d[:, :])
            nc.sync.dma_start(out=ident32, in_=idf_d[:, :])
            for idx, (c, col0, w) in enumerate(chunks):
                sl = slice(col0, col0 + w)
                ring = nc.sync if idx % 2 == 0 else nc.scalar
                ring.dma_start(out=q[:, c, sl], in_=x_d[c * P : (c + 1) * P, sl])

            def cast(idx):
                c, col0, w = chunks[idx]
                sl = slice(col0, col0 + w)
                nc.vector.tensor_copy(out=q_bf[:, c, sl], in_=q[:, c, sl])

            def pe_transpose(idx):
                c, col0, w = chunks[idx]
                kb = w // P
                tp = opsum.tile([P, w], bf16, name="tp", tag="ops")
                for kk in range(kb):
                    a = col0 + kk * P
                    nc.tensor.transpose(
                        tp[:, kk * P : (kk + 1) * P], q_bf[:, c, a : a + P], ident
                    )
                nc.vector.tensor_copy(
                    out=qT[:, col0 // P : col0 // P + kb, c, :],
                    in_=tp.rearrange("p (k v) -> p k v", v=P),
                )

            # ---- energy accumulators (one PSUM bank each) ----
            Es = [
                epsum.tile([P, C], fp32, name=f"E{i}", tag=f"E{i}", bufs=1)
                for i in range(CT)
            ]

            # E is symmetric: compute only column blocks j >= i; j < i blocks
            # are mirrored from E[j] after its accumulation completes.
            def energy(k0, k1):
                for k in range(k0, k1):
                    for i in range(CT):
                        nc.tensor.matmul(
                            Es[i][:, i * P :],
                            lhsT=qT[:, k, i, :],
                            rhs=qT[:, k, i:, :],
                            start=(k == 0),
                            stop=(k == KT - 1),
                        )

            # load phase: cast+transpose+gather per chunk, energy per k-slab
            for idx in range(12):
                cast(idx)
                pe_transpose(idx)
                if idx % CT == CT - 1:
                    g = idx // CT
                    energy(8 * g, 8 * g + 8)
            for idx in range(12, 16):
                cast(idx)
                pe_transpose(idx)
            energy(24, 28)
            for idx in range(16, 20):
                cast(idx)
                pe_transpose(idx)

            # ---- tail: staggered E completion, softmax, attT ----
            rgs = []
            for i in range(CT):
                E = Es[i]
                for k in range(28, KT):
                    nc.tensor.matmul(
                        E[:, i * P :],
                        lhsT=qT[:, k, i, :],
                        rhs=qT[:, k, i:, :],
                        start=False,
                        stop=(k == KT - 1),
                    )
                for j in range(i):
                    etmp = stats.tile([P, P], fp32, name="etmp", tag="etmp")
                    nc.vector.tensor_copy(
                        out=etmp, in_=Es[j][:, i * P : (i + 1) * P]
                    )
                    nc.tensor.transpose(E[:, j * P : (j + 1) * P], etmp, ident32)

                # softmax (unnormalized): att = exp(mn - E), s = rowsum
                mn = stats.tile([P, 1], fp32, name="mn", tag="mn")
                nc.vector.tensor_reduce(out=mn, in_=E, axis=AX, op=ALU.min)
                s = stats.tile([P, 1], fp32, name="s", tag="s")
                nc.scalar.activation(
                    out=att[:, i, :],
                    in_=E,
                    func=ACT_EXP,
                    bias=mn,
                    scale=-1.0,
                    accum_out=s,
                )
                rg = rgp.tile([P, 1], fp32, name="rg", tag="rg")
                nc.vector.reciprocal(out=rg, in_=s)
                nc.vector.tensor_mul(rg, rg, gam)
                rgs.append(rg)

                if i == 0:
                    # fast path: PE transpose of att row-block 0
                    tp = atps.tile([P, CT * P], bf16, name="atp", tag="atp")
                    for j in range(CT):
                        nc.tensor.transpose(
                            tp[:, j * P : (j + 1) * P],
                            att[:, 0, j * P : (j + 1) * P],
                            ident,
                        )
                    nc.vector.tensor_copy(
                        out=attT[:, 0, :, :],
                        in_=tp.rearrange("p (j v) -> p j v", v=P),
                    )
                else:
                    tr = nc.sync if i % 2 == 0 else nc.scalar
                    tr.dma_start_transpose(out=attT[:, i, :, :], in_=att[:, i, :])

            # ---- out = gamma/s * (att @ q) + x, 512-col chunks ----
            for i in range(CT):
                for ch in range(N // FD):
                    sl = slice(ch * FD, (ch + 1) * FD)
                    ops = opsum.tile([P, FD], fp32, name="ops", tag="ops")
                    for j in range(CT):
                        nc.tensor.matmul(
                            ops,
                            lhsT=attT[:, i, j, :],
                            rhs=q_bf[:, j, sl],
                            start=(j == 0),
                            stop=(j == CT - 1),
                        )
                    ot = outp.tile([P, FD], fp32, name="ot", tag="ot")
                    nc.vector.scalar_tensor_tensor(
                        out=ot,
                        in0=ops,
                        scalar=rgs[i],
                        in1=q[:, i, sl],
                        op0=ALU.mult,
                        op1=ALU.add,
                    )
                    st = nc.sync if (i * (N // FD) + ch) % 2 == 0 else nc.scalar
                    st.dma_start(out=o_d[i * P : (i + 1) * P, sl], in_=ot)

    nc.compile()
    return nc


def _get_nc():
    if "nc" not in _CACHE:
        _CACHE["nc"] = _build_bass()
    return _CACHE["nc"]


def run(x, gamma, **run_kwargs):
    """Run on 8 cores; returns (results_list, BassKernelResults)."""
    import ml_dtypes
    from concourse.bass_utils import run_bass_kernel_spmd

    nc = _get_nc()
    x = np.ascontiguousarray(x, dtype=np.float32)
    gamma = np.ascontiguousarray(gamma, dtype=np.float32)
    gammab = np.broadcast_to(gamma.reshape(1, 1), (P, 1)).copy()
    identb = np.eye(P, dtype=ml_dtypes.bfloat16)
    identf = np.eye(P, dtype=np.float32)
    in_maps = [
        {
            "x": np.ascontiguousarray(x[b].reshape(C, N)),
            "gammab": gammab,
            "identb": identb,
            "identf": identf,
        }
        for b in range(B)
    ]
    res = run_bass_kernel_spmd(nc, in_maps, core_ids=list(range(B)), **run_kwargs)
    out = np.stack([r["out"] for r in res.results]).reshape(B, C, H, W)
    return out, res


def kernel(x, gamma):
    out, _ = run(x, gamma)
    return out.astype(np.float32)
